# revision 11
# baseline (speedup 1.0000x reference)
"""DeepTreeLSTM Trainium2 Bass kernel (v2: all-tanh gates).

B=256 perfect binary trees (511 nodes, BFS layout), ChildSum TreeLSTM
bottom-up + MLP head. Data-parallel over trees: 32 trees per NeuronCore
x 8 cores. Device tensors use a transposed "feature-on-partition"
layout: [H (2 chunks of 128 partitions), columns], columns tree-major.

v2 key idea: sigma(x) = 0.5*(1+tanh(x/2)), with the 0.5 scales folded
into host-prepped weights and the (1+t) forms folded into fused DVE
scalar_tensor_tensor ops. Every gate nonlinearity becomes Tanh, so a
block's 6 gate chunks drain as two [P,3,512] ACT instructions from two
3-bank PSUM tiles that ping-pong against the PE (2-deep pipeline in 6
banks; f-gates use the other 2). Stored device quantities: H = 2h,
C = 2c (consumer weights pre-scaled to match).

Per-block dataflow (w<=512 parents):
  f:    PE [P,2,wc] psum -> ACT tanh(+b/2) -> DVE zf=(tf+1)*C in place
  pair: GPSIMD Ht=H_l+H_r, zsum=zf_l+zf_r; hsum from Ht (head's mean)
  iou:  PE 12 matmuls -> 2x[P,3,w] psum -> ACT tanh x2 -> t6 (bf16)
  cell: DVE zi=(ti+1)*tu; C=(zsum*0.5)+zi; ACT T=tanh(C*0.5);
        DVE H=(to+1)*T

Contract notes vs the reference: the h input is unused (shape only);
c, b_iou, b_in, b_mid, b_out are all-zero per the problem's input spec,
so the kernel drops them (only U_f_b is a live bias).
"""

import os
import sys

import ml_dtypes
import numpy as np

BFNP = ml_dtypes.bfloat16

for _p in ("/opt/trn_rl_repo", "/root/.axon_site/_ro/trn_rl_repo"):
    if os.path.isdir(_p) and _p not in sys.path:
        sys.path.insert(0, _p)

import concourse.bass as bass
import concourse.mybir as mybir
import concourse.tile as tile
from concourse import bacc
from concourse.bass_utils import run_bass_kernel_spmd

P = 128
F32 = mybir.dt.float32
BF16 = mybir.dt.bfloat16
H = 256           # hidden size (2 partition chunks)
NB = 32           # trees per core
LEAF = 256        # leaves per tree
COLS = NB * LEAF  # leaf columns per core = 8192
BLK = 512
NBLK = COLS // BLK
AF = mybir.ActivationFunctionType
OP = mybir.AluOpType

_PROG = None


def _build_program():
    nc = bacc.Bacc("TRN2", target_bir_lowering=False, debug=False,
                   num_devices=8)

    xT = nc.dram_tensor("xT", [P, 2, COLS], BF16, kind="ExternalInput")
    wiouT = nc.dram_tensor("wiouT", [P, 2, 768], BF16, kind="ExternalInput")
    uiouT = nc.dram_tensor("uiouT", [P, 2, 768], BF16, kind="ExternalInput")
    ufT = nc.dram_tensor("ufT", [P, 2, 256], BF16, kind="ExternalInput")
    ufb = nc.dram_tensor("ufb", [P, 2], F32, kind="ExternalInput")
    winT = nc.dram_tensor("winT", [P, 5, P], BF16, kind="ExternalInput")
    emoT = nc.dram_tensor("emoT", [P, NB], BF16, kind="ExternalInput")
    wmidT = nc.dram_tensor("wmidT", [P, 64], F32, kind="ExternalInput")
    woutT = nc.dram_tensor("woutT", [P, 4], F32, kind="ExternalInput")
    out_t = nc.dram_tensor("out_t", [4, NB], F32, kind="ExternalOutput")

    with tile.TileContext(nc) as tc:
        with (
            tc.tile_pool(name="wp", bufs=1) as wp,
            tc.tile_pool(name="pers", bufs=1) as pers,
        ):
            wiou_sb = wp.tile([P, 2, 768], BF16)
            uiou_sb = wp.tile([P, 2, 768], BF16)
            uf_sb = wp.tile([P, 2, 256], BF16)
            ufb_sb = wp.tile([P, 2], F32)
            win_sb = wp.tile([P, 5, P], BF16)
            emo_sb = wp.tile([P, NB], BF16)
            wmid_sb = wp.tile([P, 64], F32)
            wout_sb = wp.tile([P, 4], F32)
            for sb, dr in ((wiou_sb, wiouT), (uiou_sb, uiouT), (uf_sb, ufT),
                           (ufb_sb, ufb), (win_sb, winT), (emo_sb, emoT),
                           (wmid_sb, wmidT), (wout_sb, woutT)):
                nc.sync.dma_start(sb[:], dr[:])

            h7 = pers.tile([P, 2, NB * 128], BF16)
            c7 = pers.tile([P, 2, NB * 128], BF16)
            hsum = pers.tile([P, 2, NB], F32)
            hlast = pers.tile([P, 2, NB], F32)
            nc.vector.memset(hsum[:], 0.0)

            def iou_t6(pps, pool, rhs, w_sb, n, tag):
                """t6 = tanh(W @ rhs): 12 matmuls into two [P,3,n] psum
                tiles (3 banks each, separate tags so consecutive blocks
                ping-pong), each drained by one ACT Tanh."""
                t6 = pool.tile([P, 6, n], BF16, tag="t6", bufs=3,
                               name=f"t6_{tag}")
                for hf in range(2):
                    pg = pps.tile([P, 3, BLK], F32, tag=f"iou{hf}",
                                  name=f"pg_{tag}_{hf}")
                    for g in range(3):
                        mm = hf * 3 + g
                        for k in range(2):
                            nc.tensor.matmul(pg[:, g, :n],
                                             w_sb[:, k, mm * P:(mm + 1) * P],
                                             rhs[:, k, :],
                                             start=(k == 0), stop=(k == 1))
                    nc.scalar.activation(t6[:, 3 * hf:3 * hf + 3, :],
                                         pg[:, :, :n], AF.Tanh)
                return t6

            pending = []

            def flush_pending():
                for fn in pending:
                    fn()
                pending.clear()

            def cell_mid(pool, t6, out_c, w, d_tag, leaf=False):
                """i*u (+ c_agg already in out_c unless leaf): sigma from
                tanh via tensor_scalar (4x mode), mult/add at 2x."""
                i1 = pool.tile([P, 2, w], BF16, tag="i1", bufs=3,
                               name=f"i1_{d_tag}")
                nc.vector.tensor_scalar(i1[:], t6[:, 0:2, :], 0.5, 0.5,
                                        op0=OP.mult, op1=OP.add)
                if leaf:
                    nc.vector.tensor_mul(out_c, i1[:], t6[:, 4:6, :])
                else:
                    iu = pool.tile([P, 2, w], BF16, tag="iu", bufs=3,
                                   name=f"iu_{d_tag}")
                    nc.vector.tensor_mul(iu[:], i1[:], t6[:, 4:6, :])
                    nc.vector.tensor_add(out_c, iu[:], out_c)

            def cell_tail(pool, t6, out_h, out_c, w, d_tag):
                """T = tanh(c); h = sigma(o)*T. Deferred one block so the
                ACT/DVE queues never head-of-line block on each other."""
                ts = pool.tile([P, 2, w], BF16, tag="tb", bufs=3,
                               name=f"t_{d_tag}")
                nc.scalar.activation(ts[:], out_c, AF.Tanh)
                o1 = pool.tile([P, 2, w], BF16, tag="o1", bufs=3,
                               name=f"o1_{d_tag}")
                nc.vector.tensor_scalar(o1[:], t6[:, 2:4, :], 0.5, 0.5,
                                        op0=OP.mult, op1=OP.add)
                nc.vector.tensor_mul(out_h, o1[:], ts[:])

            def level_body(pool, pps, ch_h, ch_c, out_h, out_c, m, hsum_dst,
                           trees, d_tag):
                """One internal level (or a column range of level 7).

                ch_h/ch_c: children APs [P, 2, 2m] (bf16, stored 2h/2c);
                out_h/out_c: [P, 2, m]. hsum_dst: hsum slice [P,2,trees].
                """
                fcv = ch_c.rearrange("p k (m two) -> p k m two", two=2)
                hv = ch_h.rearrange("p k (m two) -> p k m two", two=2)
                n_j = (m + BLK - 1) // BLK
                for j in range(n_j):
                    w = min(BLK, m - j * BLK)
                    s = slice(j * BLK, j * BLK + w)
                    flush_pending()
                    # f gates + fc=sigma(f)*c in place over this block's
                    # 2w children, interleaved per parent block so the PE
                    # always has iou matmuls to fill f-psum WAR stalls
                    for jc in range((2 * w + BLK - 1) // BLK):
                        wc = min(BLK, 2 * w - jc * BLK)
                        sc = slice(2 * j * BLK + jc * BLK,
                                   2 * j * BLK + jc * BLK + wc)
                        pf = pps.tile([P, 2, BLK], F32, tag="f",
                                      name=f"pf_{d_tag}_{j}_{jc}")
                        f_sb = pool.tile([P, 2, wc], BF16, tag="fb", bufs=3,
                                         name=f"f_{d_tag}_{j}_{jc}")
                        for g in range(2):
                            for k in range(2):
                                nc.tensor.matmul(
                                    pf[:, g, :wc],
                                    uf_sb[:, k, g * P:(g + 1) * P],
                                    ch_h[:, k, sc],
                                    start=(k == 0), stop=(k == 1))
                            nc.scalar.activation(f_sb[:, g, :wc],
                                                 pf[:, g, :wc], AF.Tanh,
                                                 bias=ufb_sb[:, g:g + 1])
                        f1 = pool.tile([P, 2, wc], BF16, tag="f1", bufs=3,
                                       name=f"f1_{d_tag}_{j}_{jc}")
                        nc.vector.tensor_scalar(f1[:], f_sb[:], 0.5, 0.5,
                                                op0=OP.mult, op1=OP.add)
                        nc.vector.tensor_mul(ch_c[:, :, sc], f1[:],
                                             ch_c[:, :, sc])

                    # pair sums (gpsimd, strided reads)
                    ht = pool.tile([P, 2, w], BF16, tag="ht", bufs=3,
                                   name=f"ht_{d_tag}_{j}")
                    nc.gpsimd.tensor_add(out_c[:, :, s], fcv[:, :, s, 0],
                                         fcv[:, :, s, 1])
                    nc.gpsimd.tensor_add(ht[:], hv[:, :, s, 0],
                                         hv[:, :, s, 1])
                    # head's inner mean: summing the pair sums covers the
                    # children level; root is never summed
                    if hsum_dst is not None:
                        tj = trees // n_j
                        hs = slice(j * tj, j * tj + tj)
                        if w // tj > 1:
                            part = pool.tile([P, 2, tj], F32, tag="part",
                                             bufs=2,
                                             name=f"part_{d_tag}_{j}")
                            nc.vector.tensor_reduce(
                                part[:],
                                ht.rearrange("p k (t n) -> p k t n", t=tj),
                                axis=mybir.AxisListType.X, op=OP.add)
                            nc.gpsimd.tensor_add(hsum_dst[:, :, hs],
                                                 part[:],
                                                 hsum_dst[:, :, hs])
                        else:
                            nc.gpsimd.tensor_add(hsum_dst[:, :, hs], ht[:],
                                                 hsum_dst[:, :, hs])
                    t6 = iou_t6(pps, pool, ht[:, :, :w], uiou_sb, w,
                                f"{d_tag}_{j}")
                    cell_mid(pool, t6, out_c[:, :, s], w, f"{d_tag}_{j}")
                    pending.append(
                        lambda pool=pool, t6=t6, oh=out_h[:, :, s],
                        oc=out_c[:, :, s], w=w, tg=f"{d_tag}_{j}":
                        cell_tail(pool, t6, oh, oc, w, tg))

            with tc.tile_pool(name="pps", bufs=1, space="PSUM") as pps:
                with tc.tile_pool(name="pa", bufs=2) as pa:
                    # ---- phase A: leaves fused with level 7, 8 super-
                    # blocks of 1024 leaf cols (4 trees) ----
                    for sb in range(NBLK // 2):
                        hl = pa.tile([P, 2, 2 * BLK], BF16, tag="hl", bufs=2,
                                     name=f"hl_{sb}")
                        cl = pa.tile([P, 2, 2 * BLK], BF16, tag="cl", bufs=2,
                                     name=f"cl_{sb}")
                        for half in range(2):
                            b = 2 * sb + half
                            hs = slice(half * BLK, half * BLK + BLK)
                            xk = pa.tile([P, 2, BLK], BF16, tag="xk", bufs=4,
                                         name=f"xk_{b}")
                            nc.sync.dma_start(xk[:], xT[:, :, b * BLK:
                                                         (b + 1) * BLK])
                            flush_pending()
                            t6 = iou_t6(pps, pa, xk[:], wiou_sb, BLK,
                                        f"A{b}")
                            cell_mid(pa, t6, cl[:, :, hs], BLK, f"A{b}",
                                     leaf=True)
                            pending.append(
                                lambda t6=t6, oh=hl[:, :, hs],
                                oc=cl[:, :, hs], tg=f"A{b}":
                                cell_tail(pa, t6, oh, oc, BLK, tg))
                        # last leaf (tree-local leaf 255) of each tree
                        pending.append(
                            lambda sb=sb, hl=hl:
                            nc.vector.tensor_copy(
                                hlast[:, :, 4 * sb:4 * sb + 4],
                                hl[:, :, 255::256]))
                        # level 7 for this super-block's 512 parents
                        ps = slice(sb * BLK, sb * BLK + BLK)
                        level_body(pa, pps, hl[:], cl[:], h7[:, :, ps],
                                   c7[:, :, ps], BLK,
                                   hsum[:, :, 4 * sb:4 * sb + 4], 4, f"A{sb}")
                    flush_pending()

                    # ---- phase B: levels 6..0 over all trees ----
                h_prev, c_prev = h7, c7
                h_root = None
                with tc.tile_pool(name="pb", bufs=1) as pb:
                    for d in range(6, -1, -1):
                        m = NB * (2 ** d)
                        h_cur = pb.tile([P, 2, m], BF16, tag="hlvl",
                                        bufs=2, name=f"h_{d}")
                        c_cur = pb.tile([P, 2, m], BF16, tag="clvl",
                                        bufs=2, name=f"c_{d}")
                        level_body(pb, pps, h_prev[:, :, :2 * m],
                                   c_prev[:, :, :2 * m], h_cur[:], c_cur[:],
                                   m, hsum[:], NB, f"B{d}")
                        h_prev, c_prev = h_cur, c_cur
                        if d == 0:
                            h_root = h_cur

                    # ---- head (fp32 tail; all head biases are zero;
                    # the 1/509 inner-mean scale folded into winT) ----
                    flush_pending()
                    inner = pb.tile([P, 2, NB], BF16)
                    nc.vector.tensor_sub(inner[:], hsum[:], hlast[:])
                    y2_sb = pb.tile([P, NB], F32)
                    nc.vector.memset(y2_sb[:], 0.0)

                    pht = pps.tile([P, 2, BLK], F32, tag="f", name="p_head")
                    py1 = pht[:, 0, :NB]
                    chunks = [h_root[:, 0, :], h_root[:, 1, :],
                              inner[:, 0, :], inner[:, 1, :], emo_sb[:]]
                    for k in range(5):
                        nc.tensor.matmul(py1, win_sb[:, k, :], chunks[k],
                                         start=(k == 0), stop=(k == 4))
                    y1_sb = pb.tile([P, NB], F32)
                    nc.scalar.activation(y1_sb[:], py1, AF.Relu)
                    py2 = pht[:64, 1, :NB]
                    nc.tensor.matmul(py2, wmid_sb[:], y1_sb[:])
                    nc.scalar.activation(y2_sb[:64, :], py2, AF.Relu)
                    pht2 = pps.tile([P, 2, BLK], F32, tag="f", name="p_out")
                    po = pht2[:4, 0, :NB]
                    nc.tensor.matmul(po, wout_sb[:], y2_sb[:])
                    o_sb = pb.tile([4, NB], F32)
                    nc.scalar.activation(o_sb[:], po, AF.Sigmoid)
                    nc.sync.dma_start(out_t[:], o_sb[:])

    nc.finalize()
    return nc


def _chunked(w):
    """[K, M] host array -> [P, K//P, M] device layout (K on partitions)."""
    k, m = w.shape
    return np.ascontiguousarray(w.reshape(k // P, P, m).transpose(1, 0, 2))


def _prep_shared(W_iou, U_iou, b_iou, U_f_w, U_f_b, W_in, b_in, W_mid, b_mid,
                 W_out, b_out):
    f = np.float32
    # sigma(x)=0.5*(1+tanh(x/2)): halve the i,o and f pre-activation
    # weight rows so the device computes tanh(x/2); the 0.5/+0.5 affine
    # happens in 4x-mode DVE tensor_scalar ops. h and c stay true-valued.
    W_iou = np.asarray(W_iou, f).copy()
    W_iou[:512] *= 0.5
    U_iou = np.asarray(U_iou, f).copy()
    U_iou[:512] *= 0.5
    U_f = np.asarray(U_f_w, f) * 0.5
    ufb_h = np.ascontiguousarray((np.asarray(U_f_b, f) * 0.5
                                  ).reshape(2, P).T).astype(f)
    wiouT = _chunked(np.ascontiguousarray(W_iou.T)).astype(BFNP)
    uiouT = _chunked(np.ascontiguousarray(U_iou.T)).astype(BFNP)
    ufT = _chunked(np.ascontiguousarray(U_f.T)).astype(BFNP)
    # head: inner uses (hsum-hlast) -> fold the 1/509 mean scale.
    W_in = np.asarray(W_in, f).copy()
    W_in[:, 256:512] *= 1.0 / 509.0
    winT = np.zeros((640, P), f)
    winT[:544] = W_in.T
    winT = _chunked(winT).astype(BFNP)
    wmidT = np.ascontiguousarray(np.asarray(W_mid, f).T).astype(f)
    woutT = np.zeros((P, 4), f)
    woutT[:64] = np.asarray(W_out, f).T
    return dict(wiouT=wiouT, uiouT=uiouT, ufT=ufT, ufb=ufb_h,
                winT=winT, wmidT=wmidT, woutT=woutT)


def _run(X, emo, shared, trace=False):
    global _PROG
    if _PROG is None:
        _PROG = _build_program()
    nc = _PROG

    in_maps = []
    for cc in range(8):
        Xc = X[cc * NB:(cc + 1) * NB, 255:511, :]
        xT = Xc.transpose(2, 0, 1).reshape(256, COLS)
        xT = np.ascontiguousarray(
            xT.reshape(2, P, COLS).transpose(1, 0, 2)).astype(BFNP)
        emoT = np.zeros((P, NB), BFNP)
        emoT[:32] = emo[cc * NB:(cc + 1) * NB].T.astype(BFNP)
        in_maps.append(dict(xT=xT, emoT=emoT, **shared))

    res = None
    for attempt in range(3):
        try:
            res = run_bass_kernel_spmd(nc, in_maps, core_ids=list(range(8)),
                                       trace=trace)
            break
        except Exception:
            if attempt == 2:
                raise
    out = np.concatenate([res.results[cc]["out_t"].T for cc in range(8)],
                         axis=0)
    return np.ascontiguousarray(out.astype(np.float32)), res


def kernel(X, h, c, emo, W_iou, U_iou, b_iou, U_f_w, U_f_b,
           W_in, b_in, W_mid, b_mid, W_out, b_out, **kwargs):
    X = np.asarray(X, np.float32)
    emo = np.asarray(emo, np.float32)
    shared = _prep_shared(np.asarray(W_iou), np.asarray(U_iou),
                          np.asarray(b_iou), np.asarray(U_f_w),
                          np.asarray(U_f_b), np.asarray(W_in),
                          np.asarray(b_in), np.asarray(W_mid),
                          np.asarray(b_mid), np.asarray(W_out),
                          np.asarray(b_out))
    out, _ = _run(X, emo, shared)
    return out


# revision 13
# speedup vs baseline: 1.1066x; 1.1066x over previous
"""DeepTreeLSTM Trainium2 Bass kernel (v2: all-tanh gates).

B=256 perfect binary trees (511 nodes, BFS layout), ChildSum TreeLSTM
bottom-up + MLP head. Data-parallel over trees: 32 trees per NeuronCore
x 8 cores. Device tensors use a transposed "feature-on-partition"
layout: [H (2 chunks of 128 partitions), columns], columns tree-major.

v2 key idea: sigma(x) = 0.5*(1+tanh(x/2)), with the 0.5 scales folded
into host-prepped weights and the (1+t) forms folded into fused DVE
scalar_tensor_tensor ops. Every gate nonlinearity becomes Tanh, so a
block's 6 gate chunks drain as two [P,3,512] ACT instructions from two
3-bank PSUM tiles that ping-pong against the PE (2-deep pipeline in 6
banks; f-gates use the other 2). Stored device quantities: H = 2h,
C = 2c (consumer weights pre-scaled to match).

Per-block dataflow (w<=512 parents):
  f:    PE [P,2,wc] psum -> ACT tanh(+b/2) -> DVE zf=(tf+1)*C in place
  pair: GPSIMD Ht=H_l+H_r, zsum=zf_l+zf_r; hsum from Ht (head's mean)
  iou:  PE 12 matmuls -> 2x[P,3,w] psum -> ACT tanh x2 -> t6 (bf16)
  cell: DVE zi=(ti+1)*tu; C=(zsum*0.5)+zi; ACT T=tanh(C*0.5);
        DVE H=(to+1)*T

Contract notes vs the reference: the h input is unused (shape only);
c, b_iou, b_in, b_mid, b_out are all-zero per the problem's input spec,
so the kernel drops them (only U_f_b is a live bias).
"""

import os
import sys

import ml_dtypes
import numpy as np

BFNP = ml_dtypes.bfloat16

for _p in ("/opt/trn_rl_repo", "/root/.axon_site/_ro/trn_rl_repo"):
    if os.path.isdir(_p) and _p not in sys.path:
        sys.path.insert(0, _p)

import concourse.bass as bass
import concourse.mybir as mybir
import concourse.tile as tile
from concourse import bacc
from concourse.bass_utils import run_bass_kernel_spmd

P = 128
F32 = mybir.dt.float32
BF16 = mybir.dt.bfloat16
H = 256           # hidden size (2 partition chunks)
NB = 32           # trees per core
LEAF = 256        # leaves per tree
COLS = NB * LEAF  # leaf columns per core = 8192
BLK = 512
NBLK = COLS // BLK
AF = mybir.ActivationFunctionType
OP = mybir.AluOpType

_PROG = None


def _build_program():
    nc = bacc.Bacc("TRN2", target_bir_lowering=False, debug=False,
                   num_devices=8)

    xT = nc.dram_tensor("xT", [P, 2, COLS], BF16, kind="ExternalInput")
    wiouT = nc.dram_tensor("wiouT", [P, 2, 768], BF16, kind="ExternalInput")
    uiouT = nc.dram_tensor("uiouT", [P, 2, 768], BF16, kind="ExternalInput")
    ufT = nc.dram_tensor("ufT", [P, 2, 256], BF16, kind="ExternalInput")
    ufb = nc.dram_tensor("ufb", [P, 2], F32, kind="ExternalInput")
    winT = nc.dram_tensor("winT", [P, 5, P], BF16, kind="ExternalInput")
    emoT = nc.dram_tensor("emoT", [P, NB], BF16, kind="ExternalInput")
    wmidT = nc.dram_tensor("wmidT", [P, 64], F32, kind="ExternalInput")
    woutT = nc.dram_tensor("woutT", [P, 4], F32, kind="ExternalInput")
    out_t = nc.dram_tensor("out_t", [4, NB], F32, kind="ExternalOutput")

    with tile.TileContext(nc) as tc:
        with (
            tc.tile_pool(name="wp", bufs=1) as wp,
            tc.tile_pool(name="pers", bufs=1) as pers,
        ):
            wiou_sb = wp.tile([P, 2, 768], BF16)
            uiou_sb = wp.tile([P, 2, 768], BF16)
            uf_sb = wp.tile([P, 2, 256], BF16)
            ufb_sb = wp.tile([P, 2], F32)
            win_sb = wp.tile([P, 5, P], BF16)
            emo_sb = wp.tile([P, NB], BF16)
            wmid_sb = wp.tile([P, 64], F32)
            wout_sb = wp.tile([P, 4], F32)
            for sb, dr in ((wiou_sb, wiouT), (uiou_sb, uiouT), (uf_sb, ufT),
                           (ufb_sb, ufb), (win_sb, winT), (emo_sb, emoT),
                           (wmid_sb, wmidT), (wout_sb, woutT)):
                nc.sync.dma_start(sb[:], dr[:])

            h7 = pers.tile([P, 2, NB * 128], BF16)
            c7 = pers.tile([P, 2, NB * 128], BF16)
            hsum = pers.tile([P, 2, NB], F32)
            hlast = pers.tile([P, 2, NB], F32)
            nc.vector.memset(hsum[:], 0.0)

            def iou_t6(pps, pool, rhs, w_sb, n, tag):
                """t6 = tanh(W @ rhs): 12 matmuls into two [P,3,n] psum
                tiles (3 banks each, separate tags so consecutive blocks
                ping-pong), each drained by one ACT Tanh."""
                t6 = pool.tile([P, 6, n], BF16, tag="t6", bufs=3,
                               name=f"t6_{tag}")
                for hf in range(2):
                    pg = pps.tile([P, 3, BLK], F32, tag=f"iou{hf}",
                                  name=f"pg_{tag}_{hf}")
                    for g in range(3):
                        mm = hf * 3 + g
                        for k in range(2):
                            nc.tensor.matmul(pg[:, g, :n],
                                             w_sb[:, k, mm * P:(mm + 1) * P],
                                             rhs[:, k, :],
                                             start=(k == 0), stop=(k == 1))
                    nc.scalar.activation(t6[:, 3 * hf:3 * hf + 3, :],
                                         pg[:, :, :n], AF.Tanh)
                return t6

            pending = []

            def flush_pending():
                for fn in pending:
                    fn()
                pending.clear()

            def cell_mid(pool, t6, out_c, w, d_tag, leaf=False):
                """i*u (+ c_agg already in out_c unless leaf): sigma from
                tanh via tensor_scalar (4x mode), mult/add at 2x."""
                i1 = pool.tile([P, 2, w], BF16, tag="i1", bufs=3,
                               name=f"i1_{d_tag}")
                nc.vector.tensor_scalar(i1[:], t6[:, 0:2, :], 0.5, 0.5,
                                        op0=OP.mult, op1=OP.add)
                if leaf:
                    nc.vector.tensor_mul(out_c, i1[:], t6[:, 4:6, :])
                else:
                    iu = pool.tile([P, 2, w], BF16, tag="iu", bufs=3,
                                   name=f"iu_{d_tag}")
                    nc.vector.tensor_mul(iu[:], i1[:], t6[:, 4:6, :])
                    nc.vector.tensor_add(out_c, iu[:], out_c)

            def cell_tail(pool, t6, out_h, out_c, w, d_tag):
                """T = tanh(c); h = sigma(o)*T. Deferred one block so the
                ACT/DVE queues never head-of-line block on each other."""
                ts = pool.tile([P, 2, w], BF16, tag="tb", bufs=3,
                               name=f"t_{d_tag}")
                nc.scalar.activation(ts[:], out_c, AF.Tanh)
                o1 = pool.tile([P, 2, w], BF16, tag="o1", bufs=3,
                               name=f"o1_{d_tag}")
                nc.vector.tensor_scalar(o1[:], t6[:, 2:4, :], 0.5, 0.5,
                                        op0=OP.mult, op1=OP.add)
                nc.vector.tensor_mul(out_h, o1[:], ts[:])

            def level_body(pool, pps, ch_h, ch_c, out_h, out_c, m, hsum_dst,
                           trees, d_tag, flush=True):
                """One internal level (or a column range of level 7).

                ch_h/ch_c: children APs [P, 2, 2m] (bf16);
                out_h/out_c: [P, 2, m]. hsum_dst: hsum slice [P,2,trees].
                """
                fcv = ch_c.rearrange("p k (m two) -> p k m two", two=2)
                hv = ch_h.rearrange("p k (m two) -> p k m two", two=2)
                n_j = (m + BLK - 1) // BLK
                for j in range(n_j):
                    w = min(BLK, m - j * BLK)
                    s = slice(j * BLK, j * BLK + w)
                    if flush:
                        flush_pending()
                    # f gates + fc=sigma(f)*c in place over this block's
                    # 2w children, interleaved per parent block so the PE
                    # always has iou matmuls to fill f-psum WAR stalls
                    for jc in range((2 * w + BLK - 1) // BLK):
                        wc = min(BLK, 2 * w - jc * BLK)
                        sc = slice(2 * j * BLK + jc * BLK,
                                   2 * j * BLK + jc * BLK + wc)
                        pf = pps.tile([P, 2, BLK], F32, tag="f",
                                      name=f"pf_{d_tag}_{j}_{jc}")
                        f_sb = pool.tile([P, 2, wc], BF16, tag="fb", bufs=3,
                                         name=f"f_{d_tag}_{j}_{jc}")
                        for g in range(2):
                            for k in range(2):
                                nc.tensor.matmul(
                                    pf[:, g, :wc],
                                    uf_sb[:, k, g * P:(g + 1) * P],
                                    ch_h[:, k, sc],
                                    start=(k == 0), stop=(k == 1))
                            nc.scalar.activation(f_sb[:, g, :wc],
                                                 pf[:, g, :wc], AF.Tanh,
                                                 bias=ufb_sb[:, g:g + 1])
                        f1 = pool.tile([P, 2, wc], BF16, tag="f1", bufs=3,
                                       name=f"f1_{d_tag}_{j}_{jc}")
                        nc.vector.tensor_scalar(f1[:], f_sb[:], 0.5, 0.5,
                                                op0=OP.mult, op1=OP.add)
                        nc.vector.tensor_mul(ch_c[:, :, sc], f1[:],
                                             ch_c[:, :, sc])

                    # pair sums (gpsimd, strided reads)
                    ht = pool.tile([P, 2, w], BF16, tag="ht", bufs=3,
                                   name=f"ht_{d_tag}_{j}")
                    nc.gpsimd.tensor_add(out_c[:, :, s], fcv[:, :, s, 0],
                                         fcv[:, :, s, 1])
                    nc.gpsimd.tensor_add(ht[:], hv[:, :, s, 0],
                                         hv[:, :, s, 1])
                    # head's inner mean: summing the pair sums covers the
                    # children level; root is never summed
                    if hsum_dst is not None:
                        tj = trees // n_j
                        hs = slice(j * tj, j * tj + tj)
                        if w // tj > 1:
                            part = pool.tile([P, 2, tj], F32, tag="part",
                                             bufs=2,
                                             name=f"part_{d_tag}_{j}")
                            nc.vector.tensor_reduce(
                                part[:],
                                ht.rearrange("p k (t n) -> p k t n", t=tj),
                                axis=mybir.AxisListType.X, op=OP.add)
                            nc.gpsimd.tensor_add(hsum_dst[:, :, hs],
                                                 part[:],
                                                 hsum_dst[:, :, hs])
                        else:
                            nc.gpsimd.tensor_add(hsum_dst[:, :, hs], ht[:],
                                                 hsum_dst[:, :, hs])
                    t6 = iou_t6(pps, pool, ht[:, :, :w], uiou_sb, w,
                                f"{d_tag}_{j}")
                    cell_mid(pool, t6, out_c[:, :, s], w, f"{d_tag}_{j}")
                    pending.append(
                        lambda pool=pool, t6=t6, oh=out_h[:, :, s],
                        oc=out_c[:, :, s], w=w, tg=f"{d_tag}_{j}":
                        cell_tail(pool, t6, oh, oc, w, tg))

            with tc.tile_pool(name="pps", bufs=1, space="PSUM") as pps:
                with tc.tile_pool(name="pa", bufs=2) as pa:
                    # ---- phase A: 8 super-blocks of 1024 leaf cols
                    # (4 trees), software-pipelined one stage deep: the
                    # level-7 body of super-block sb-1 interleaves with
                    # the leaves of sb so its matmuls only touch
                    # stage-old data and the PE never drains ----
                    hls, cls = {}, {}

                    def leaf_emit(sb):
                        hl = pa.tile([P, 2, 2 * BLK], BF16, tag="hl",
                                     bufs=2, name=f"hl_{sb}")
                        cl = pa.tile([P, 2, 2 * BLK], BF16, tag="cl",
                                     bufs=2, name=f"cl_{sb}")
                        hls[sb], cls[sb] = hl, cl
                        for half in range(2):
                            b = 2 * sb + half
                            hs = slice(half * BLK, half * BLK + BLK)
                            xk = pa.tile([P, 2, BLK], BF16, tag="xk", bufs=4,
                                         name=f"xk_{b}")
                            nc.sync.dma_start(xk[:], xT[:, :, b * BLK:
                                                         (b + 1) * BLK])
                            flush_pending()
                            t6 = iou_t6(pps, pa, xk[:], wiou_sb, BLK,
                                        f"A{b}")
                            cell_mid(pa, t6, cl[:, :, hs], BLK, f"A{b}",
                                     leaf=True)
                            pending.append(
                                lambda t6=t6, oh=hl[:, :, hs],
                                oc=cl[:, :, hs], tg=f"A{b}":
                                cell_tail(pa, t6, oh, oc, BLK, tg))
                        # last leaf (tree-local leaf 255) of each tree
                        pending.append(
                            lambda sb=sb, hl=hl:
                            nc.vector.tensor_copy(
                                hlast[:, :, 4 * sb:4 * sb + 4],
                                hl[:, :, 255::256]))

                    for sb in range(NBLK // 2 + 1):
                        if sb < NBLK // 2:
                            leaf_emit(sb)
                        if sb >= 1:
                            pv = sb - 1
                            ps = slice(pv * BLK, pv * BLK + BLK)
                            level_body(pa, pps, hls[pv][:], cls[pv][:],
                                       h7[:, :, ps], c7[:, :, ps], BLK,
                                       hsum[:, :, 4 * pv:4 * pv + 4], 4,
                                       f"A{pv}", flush=False)
                    flush_pending()

                    # ---- phase B: levels 6..0 over all trees ----
                h_prev, c_prev = h7, c7
                h_root = None
                with tc.tile_pool(name="pb", bufs=1) as pb:
                    for d in range(6, -1, -1):
                        m = NB * (2 ** d)
                        h_cur = pb.tile([P, 2, m], BF16, tag="hlvl",
                                        bufs=2, name=f"h_{d}")
                        c_cur = pb.tile([P, 2, m], BF16, tag="clvl",
                                        bufs=2, name=f"c_{d}")
                        level_body(pb, pps, h_prev[:, :, :2 * m],
                                   c_prev[:, :, :2 * m], h_cur[:], c_cur[:],
                                   m, hsum[:], NB, f"B{d}")
                        h_prev, c_prev = h_cur, c_cur
                        if d == 0:
                            h_root = h_cur

                    # ---- head (fp32 tail; all head biases are zero;
                    # the 1/509 inner-mean scale folded into winT) ----
                    flush_pending()
                    inner = pb.tile([P, 2, NB], BF16)
                    nc.vector.tensor_sub(inner[:], hsum[:], hlast[:])
                    y2_sb = pb.tile([P, NB], F32)
                    nc.vector.memset(y2_sb[:], 0.0)

                    pht = pps.tile([P, 2, BLK], F32, tag="f", name="p_head")
                    py1 = pht[:, 0, :NB]
                    chunks = [h_root[:, 0, :], h_root[:, 1, :],
                              inner[:, 0, :], inner[:, 1, :], emo_sb[:]]
                    for k in range(5):
                        nc.tensor.matmul(py1, win_sb[:, k, :], chunks[k],
                                         start=(k == 0), stop=(k == 4))
                    y1_sb = pb.tile([P, NB], F32)
                    nc.scalar.activation(y1_sb[:], py1, AF.Relu)
                    py2 = pht[:64, 1, :NB]
                    nc.tensor.matmul(py2, wmid_sb[:], y1_sb[:])
                    nc.scalar.activation(y2_sb[:64, :], py2, AF.Relu)
                    pht2 = pps.tile([P, 2, BLK], F32, tag="f", name="p_out")
                    po = pht2[:4, 0, :NB]
                    nc.tensor.matmul(po, wout_sb[:], y2_sb[:])
                    o_sb = pb.tile([4, NB], F32)
                    nc.scalar.activation(o_sb[:], po, AF.Sigmoid)
                    nc.sync.dma_start(out_t[:], o_sb[:])

    nc.finalize()
    return nc


def _chunked(w):
    """[K, M] host array -> [P, K//P, M] device layout (K on partitions)."""
    k, m = w.shape
    return np.ascontiguousarray(w.reshape(k // P, P, m).transpose(1, 0, 2))


def _prep_shared(W_iou, U_iou, b_iou, U_f_w, U_f_b, W_in, b_in, W_mid, b_mid,
                 W_out, b_out):
    f = np.float32
    # sigma(x)=0.5*(1+tanh(x/2)): halve the i,o and f pre-activation
    # weight rows so the device computes tanh(x/2); the 0.5/+0.5 affine
    # happens in 4x-mode DVE tensor_scalar ops. h and c stay true-valued.
    W_iou = np.asarray(W_iou, f).copy()
    W_iou[:512] *= 0.5
    U_iou = np.asarray(U_iou, f).copy()
    U_iou[:512] *= 0.5
    U_f = np.asarray(U_f_w, f) * 0.5
    ufb_h = np.ascontiguousarray((np.asarray(U_f_b, f) * 0.5
                                  ).reshape(2, P).T).astype(f)
    wiouT = _chunked(np.ascontiguousarray(W_iou.T)).astype(BFNP)
    uiouT = _chunked(np.ascontiguousarray(U_iou.T)).astype(BFNP)
    ufT = _chunked(np.ascontiguousarray(U_f.T)).astype(BFNP)
    # head: inner uses (hsum-hlast) -> fold the 1/509 mean scale.
    W_in = np.asarray(W_in, f).copy()
    W_in[:, 256:512] *= 1.0 / 509.0
    winT = np.zeros((640, P), f)
    winT[:544] = W_in.T
    winT = _chunked(winT).astype(BFNP)
    wmidT = np.ascontiguousarray(np.asarray(W_mid, f).T).astype(f)
    woutT = np.zeros((P, 4), f)
    woutT[:64] = np.asarray(W_out, f).T
    return dict(wiouT=wiouT, uiouT=uiouT, ufT=ufT, ufb=ufb_h,
                winT=winT, wmidT=wmidT, woutT=woutT)


def _run(X, emo, shared, trace=False):
    global _PROG
    if _PROG is None:
        _PROG = _build_program()
    nc = _PROG

    in_maps = []
    for cc in range(8):
        Xc = X[cc * NB:(cc + 1) * NB, 255:511, :]
        xT = Xc.transpose(2, 0, 1).reshape(256, COLS)
        xT = np.ascontiguousarray(
            xT.reshape(2, P, COLS).transpose(1, 0, 2)).astype(BFNP)
        emoT = np.zeros((P, NB), BFNP)
        emoT[:32] = emo[cc * NB:(cc + 1) * NB].T.astype(BFNP)
        in_maps.append(dict(xT=xT, emoT=emoT, **shared))

    res = None
    for attempt in range(3):
        try:
            res = run_bass_kernel_spmd(nc, in_maps, core_ids=list(range(8)),
                                       trace=trace)
            break
        except Exception:
            if attempt == 2:
                raise
    out = np.concatenate([res.results[cc]["out_t"].T for cc in range(8)],
                         axis=0)
    return np.ascontiguousarray(out.astype(np.float32)), res


def kernel(X, h, c, emo, W_iou, U_iou, b_iou, U_f_w, U_f_b,
           W_in, b_in, W_mid, b_mid, W_out, b_out, **kwargs):
    X = np.asarray(X, np.float32)
    emo = np.asarray(emo, np.float32)
    shared = _prep_shared(np.asarray(W_iou), np.asarray(U_iou),
                          np.asarray(b_iou), np.asarray(U_f_w),
                          np.asarray(U_f_b), np.asarray(W_in),
                          np.asarray(b_in), np.asarray(W_mid),
                          np.asarray(b_mid), np.asarray(W_out),
                          np.asarray(b_out))
    out, _ = _run(X, emo, shared)
    return out
